# revision 56
# baseline (speedup 1.0000x reference)
"""Trainium2 Bass kernel for nn_AffinityPropagate2 (8-iteration dual-dilation
affinity propagation with per-pixel softmax kernels).

Contract: kernel(**inputs) takes FULL numpy inputs
    guided1 [4,9,352,1216] f32, guided2 [4,9,352,1216] f32,
    fuse    [4,2,352,1216] f32, x [4,1,352,1216] f32
and returns the FULL output [4,1,352,1216] f32.

Strategy (8 NeuronCores, SPMD, no cross-core communication):
  - Shard: core c = (batch b = c//2, H-half = c%2). Each core owns 176 output
    rows plus a one-sided ghost zone that shrinks 2 rows per iteration;
    half-1 shards are row-flipped on the host so one SPMD program serves
    all 8 cores.
  - On-chip layout: W padded 1216->1280, 128 column strips of 10
    (partition = strip); H in the free dim. 2 halo columns per side,
    refreshed per iteration via TensorE permutation matmuls.
  - The host pre-permutes the 9 tap planes into tap-GROUP order so each
    fused group is plane-contiguous:
      w1: [A: dh=-1,0,+1 @ dw=0] [ODD-: @ dw=-1] [ODD+: @ dw=+1]
      w2: [B: dh=-2] [C: dh=+2] [D: dh=0,dw=+-2] [center]
  - Softmax folds into 17 per-tap weight planes (dil1+dil2 centers merged
    into w1[1]); zero-padding emulated by zero fuse pad columns.
  - Steady iterations are row-block pipelined: every tap group is split at
    the PSUM-chunk row boundaries (with reach slivers), the DVE walks all
    groups row-block by row-block (14 planes; the Pool/GPSIMD carries the
    3-plane C group), and the PE sums each 48-row chunk of all 17 product
    planes as soon as that row block lands, so the chunk-0 copy-out
    (ScalarE) and the next iteration's first DVE work start mid-iteration
    (the DVE runs gap-free across all 7 steady iterations). Compute dtype
    fp16 (DVE 2x mode).
"""

import numpy as np

# ---------------------------------------------------------------- geometry

def make_geom(B=4, H=352, W=1216, SW=10, NS=128, PT=8, dt_name="float16"):
    HH = H // 2
    g = dict(
        B=B, H=H, W=W, SW=SW, NS=NS, PT=PT, dt_name=dt_name,
        Wp=NS * SW,
        HH=HH,
        RW=HH + 2 * (PT - 1),      # weight rows per shard (incl. ghost)
        RXL=HH + 2 * PT,           # x rows loaded per shard
        SWH=SW + 4,                # strip width incl. 2+2 halo cols
    )
    g["RX"] = g["RXL"] + 4         # x rows incl. 2+2 zero-pad rows
    assert g["Wp"] >= W and NS <= 128
    assert g["RW"] * 2 <= 512, "halo matmul free dim must fit one PSUM bank"
    return g


# ---------------------------------------------------------------- device IR

def emit(tc, outs, ins, g):
    """Emit the SPMD per-core program into TileContext tc.

    ins: dict of DRAM APs: g1 [9,NS,RW,SW], g2 [9,NS,RW,SW] (tap-group
    plane order), fz [2,NS,RW,SW], x0 [NS,RX,SWH], pl/pr/pi [NS,NS]
    outs: y [NS,HH,SW] compute dtype
    """
    import concourse.mybir as mybir
    import concourse.bass as bass_mod

    nc = tc.nc
    NS, SW, SWH, RW, RX, HH, PT = (
        g["NS"], g["SW"], g["SWH"], g["RW"], g["RX"], g["HH"], g["PT"])
    DT = getattr(mybir.dt, g["dt_name"])
    F32 = mybir.dt.float32
    EXP = mybir.ActivationFunctionType.Exp

    g1, g2, fz, x0, pl, pr = (ins[k] for k in ("g1", "g2", "fz", "x0", "pl", "pr"))
    y = outs["y"]

    from contextlib import ExitStack
    ctx = tc.nc._emit_ctx = ExitStack()  # keep pools open until trace ends
    pool = ctx.enter_context(tc.tile_pool(name="main", bufs=1))
    psp = ctx.enter_context(tc.tile_pool(name="ps", bufs=2, space="PSUM"))

    w1 = pool.tile([NS, 9, RW, SW], DT, name="w1", tag="w1")
    w2 = pool.tile([NS, 9, RW, SW], DT, name="w2", tag="w2")
    fg = pool.tile([NS, 2, RW, SW], DT, name="fg", tag="fg")
    xb = [pool.tile([NS, RX, SWH], DT, name=f"xb{i}", tag=f"xb{i}") for i in range(2)]
    # 17 product planes (all live across a pipelined iteration); slots 9:12
    # double as iteration-0's acc/tm2/tmp scratch
    p3 = pool.tile([NS, 17, RW, SW], DT, name="p3", tag="p3")
    r32 = pool.tile([NS, RW, SW], F32, name="r32", tag="r32")
    plt = pool.tile([NS, NS], DT, name="plt", tag="plt")
    prt = pool.tile([NS, NS], DT, name="prt", tag="prt")
    pit = pool.tile([NS, NS], DT, name="pit", tag="pit")
    yc = pool.tile([NS, HH, SW], DT, name="yc", tag="yc")
    acc = p3[:, 9]
    tm2 = p3[:, 10]
    tmp = p3[:, 11]
    CH = 48
    NCH = (RW + CH - 1) // CH
    assert NCH + NCH <= 8 and CH * SW <= 512
    pacc = [psp.tile([NS, CH, SW], F32, name=f"pacc{i}", tag=f"pacc{i}", bufs=1)
            for i in range(NCH)]
    psn = [psp.tile([NS, CH, SW], F32, name=f"psn{i}", tag=f"psn{i}", bufs=1)
           for i in range(NCH)]

    def psn_halo(i, n):
        # [NS, n, 2] f32 view into psn[i]'s bank for the halo matmuls
        base = psn[i][:, 0:1, 0:1]
        return bass_mod.AP(tensor=base.tensor, offset=base.offset,
                           ap=[base.ap[0], [2, n], [1, 2]])

    # ---- loads paced for the ScalarE exp chain (the startup gate): the
    # guided planes stream back-to-back so the exps never starve; perms
    # right before the first psn matmul needs them, x0 before the first
    # stencil mult, fuse before the fg normalizer multiplies
    for k in range(3):
        nc.sync.dma_start(out=w1[:, k], in_=g1[k])
    nc.sync.dma_start(out=plt, in_=pl)
    nc.sync.dma_start(out=prt, in_=pr)
    nc.sync.dma_start(out=pit, in_=ins["pi"])
    nc.sync.dma_start(out=xb[0], in_=x0)
    for k in range(3, 9):
        nc.sync.dma_start(out=w1[:, k], in_=g1[k])
    for k in range(9):
        nc.sync.dma_start(out=w2[:, k], in_=g2[k])
    nc.sync.dma_start(out=fg[:, 0], in_=fz[0])
    nc.sync.dma_start(out=fg[:, 1], in_=fz[1])

    nc.vector.memset(xb[1][:, 0:2, :], 0.0)

    PREP = g.get("PREP_LEVEL", 3)  # perf decomposition only

    def chunks(Rt):
        return [(ci * CH, min(CH, Rt - ci * CH))
                for ci in range((Rt + CH - 1) // CH)]

    # softmax normalizer: fg[s] <- f_s / sum_k exp(g_s[k]). The 9-plane sum
    # rides TensorE identity-matmuls on the psn chunks; the DVE reads the
    # fp32 sums straight out of PSUM for the reciprocal.
    def norm_chain(s, wt):
        for j in range(9):
            for ci, (r0, rows) in enumerate(chunks(RW)):
                nc.tensor.matmul(
                    psn[ci][:, 0:rows], pit, wt[:, j, r0:r0 + rows, :],
                    start=(j == 0), stop=(j == 8))
        # ~51-ULP fp32 reciprocal: far below the fp16 pipeline noise floor
        for ci, (r0, rows) in enumerate(chunks(RW)):
            nc.vector.reciprocal_approx_fast(out=r32[:, r0:r0 + rows, :],
                                             in_=psn[ci][:, 0:rows])
        nc.vector.tensor_mul(fg[:, s], fg[:, s], r32)

    def tap_src(dh, dw, Rt, xin):
        return xin[:, 2 + dh:2 + dh + Rt, 2 + dw:2 + dw + SW]

    def with_dims(base, dims):
        """insert extra leading free dims [step, count] into a sliced AP"""
        return bass_mod.AP(tensor=base.tensor, offset=base.offset,
                           ap=[base.ap[0], *dims, *base.ap[1:]])

    def x_rows(xsrc, row0, col0, dims, a, b):
        base = xsrc[:, row0 + a:row0 + b, col0:col0 + SW]
        return with_dims(base, dims)

    def pieces(Rt, h):
        """row pieces split at the PSUM-chunk boundaries, with h-row reach
        slivers before each boundary so chunk c's sums never wait on rows
        beyond what copy-out chunks 0..c (and the halo halves) provide"""
        bs = {0, Rt}
        for c in range(1, (Rt + CH - 1) // CH):
            for b in (c * CH - h, c * CH):
                if 0 < b < Rt:
                    bs.add(b)
        bs = sorted(bs)
        return list(zip(bs[:-1], bs[1:]))

    # fused tap-group multiplies — ONE tensor op per 2/3-tap group per row
    # piece. Plane layout is group-contiguous (host pre-permuted):
    #   w1[0:3] A: dil1 dw=0 (w1[1] = merged center)  -> p3[0:3]   DVE
    #   w1[3:6] ODD-: dw=-1 -> p3[3:6]   w1[6:9] ODD+: dw=+1 -> p3[6:9] DVE
    #   w2[0:3] B: dil2 dh=-2 -> p3[9:12]                          DVE
    #   w2[3:6] C: dil2 dh=+2 -> p3[12:15]                         Pool
    #   w2[6:8] D: dil2 dh=0 dw=-+2 -> p3[15:17]                   DVE
    #   w2[8] dil2 center (iteration 0 only; merged into w1[1] after)
    def mul_A(a, b, xin):
        nc.vector.tensor_mul(
            p3[:, 0:3, a:b, :], x_rows(xin, 1, 2, [[SWH, 3]], a, b),
            w1[:, 0:3, a:b, :])

    def mul_ODD(a, b, xin, s, dw):
        nc.vector.tensor_mul(
            p3[:, s:s + 3, a:b, :],
            x_rows(xin, 1, 2 + dw, [[SWH, 3]], a, b),
            w1[:, s:s + 3, a:b, :])

    def mul_B(a, b, xin, s=9):
        nc.vector.tensor_mul(
            p3[:, s:s + 3, a:b, :],
            x_rows(xin, 0, 0, [[2, 3]], a, b), w2[:, 0:3, a:b, :])

    def mul_C(a, b, xin, s=12):  # Pool
        nc.gpsimd.tensor_mul(
            p3[:, s:s + 3, a:b, :],
            x_rows(xin, 4, 0, [[2, 3]], a, b), w2[:, 3:6, a:b, :])

    def mul_C2(a, b, xin, s=12):  # DVE variant (iteration 0)
        nc.vector.tensor_mul(
            p3[:, s:s + 3, a:b, :],
            x_rows(xin, 4, 0, [[2, 3]], a, b), w2[:, 3:6, a:b, :])

    def mul_D(a, b, xin, s=15):
        # first row piece on the Pool: evens DVE (14 planes) vs Pool (3)
        eng = nc.gpsimd if b <= CH else nc.vector
        eng.tensor_mul(
            p3[:, s:s + 2, a:b, :],
            x_rows(xin, 2, 0, [[4, 2]], a, b), w2[:, 6:8, a:b, :])

    def pool_center(Rt, xin, s=8):
        nc.gpsimd.tensor_mul(
            p3[:, s, 0:Rt, :], tap_src(0, 0, Rt, xin), w2[:, 8, 0:Rt, :])

    def mm_sum(s, n, Rt, first, last):
        # plane-major accumulation (iteration 0 only)
        for j in range(n):
            for ci, (r0, rows) in enumerate(chunks(Rt)):
                nc.tensor.matmul(
                    pacc[ci][:, 0:rows], pit,
                    p3[:, s + j, r0:r0 + rows, :],
                    start=(first and j == 0), stop=(last and j == n - 1))

    def mm_copyout_dve(Rt, dst):
        # PSUM f32 -> fp16 SBUF on the DVE (keeps ScalarE free for exps)
        for ci, (r0, rows) in enumerate(chunks(Rt)):
            nc.vector.tensor_copy(out=dst[:, r0:r0 + rows, :],
                                  in_=pacc[ci][:, 0:rows])

    def mm_copyout(Rt, dst_rows_of):
        for ci, (r0, rows) in enumerate(chunks(Rt)):
            nc.scalar.copy(out=dst_rows_of(r0, rows), in_=pacc[ci][:, 0:rows])

    HB = 2 * CH  # halo half boundary

    def halo_half(xout, r0, r1, bi):
        # refresh halo cols for interior rows [r0, r1): left halo cols
        # [0:2) <- neighbor p-1 cols [SW:SW+2) via TensorE permutation
        # matmul (the only cross-partition path in the loop)
        n = r1 - r0
        psl = psn_halo(bi, n)
        nc.tensor.matmul(psl, plt, xout[:, 2 + r0:2 + r1, SW:SW + 2],
                         start=True, stop=True)
        nc.scalar.copy(out=xout[:, 2 + r0:2 + r1, 0:2], in_=psl)
        psr = psn_halo(bi + 1, n)
        nc.tensor.matmul(psr, prt, xout[:, 2 + r0:2 + r1, 2:4],
                         start=True, stop=True)
        nc.scalar.copy(out=xout[:, 2 + r0:2 + r1, SW + 2:SW + 4], in_=psr)

    def halo_refresh(xout, Rt):
        halo_half(xout, 0, min(HB, Rt), 0)
        if Rt > HB:
            halo_half(xout, HB, Rt, 2)

    def bcast(plane_ap, n):
        # broadcast one [NS, rows, SW] plane over n planes via a 0-stride dim
        return bass_mod.AP(tensor=plane_ap.tensor, offset=plane_ap.offset,
                           ap=[plane_ap.ap[0], [0, n], plane_ap.ap[1],
                               plane_ap.ap[2]])

    PT = g.get("PTE", PT)  # emit fewer iterations (perf decomposition only)

    # ---- iteration 0, group-major, interleaved with the load/exp stream.
    # Runs on raw exp planes with per-stencil accumulators:
    #   x1 = acc1*G1 + acc2*G2   (associativity of the softmax fold)
    if PT >= 1:
        Rt = RW
        xin, xout = xb[0], xb[1]
        # g1 phase: DVE takes A + ODD-, Pool takes ODD+ (the Pool is
        # otherwise idle until the w2 exps land)
        for j in range(3):
            nc.scalar.activation(out=w1[:, 3 * j:3 * j + 3],
                                 in_=w1[:, 3 * j:3 * j + 3], func=EXP)
        # norm1 first in the PE stream: its 9-plane psn sum accumulates as
        # the exps land, so only ~3 planes remain after the last exp
        if PREP >= 2:
            norm_chain(0, w1)
        mul_A(0, Rt, xin)
        mm_sum(0, 3, Rt, True, False)
        mul_ODD(0, Rt, xin, 3, -1)
        mm_sum(3, 3, Rt, False, False)
        nc.gpsimd.tensor_mul(
            p3[:, 6:9, 0:Rt, :], x_rows(xin, 1, 3, [[SWH, 3]], 0, Rt),
            w1[:, 6:9, 0:Rt, :])
        mm_sum(6, 3, Rt, False, True)
        # dil1 combine on the DVE's idle window (the ScalarE is mid-exps)
        mm_copyout_dve(Rt, acc)
        nc.vector.tensor_mul(acc, acc, fg[:, 0])
        # w1 folds need only fg0; A/ODD- planes on the DVE (it idles while
        # the w2 exps run), ODD+ planes on the Pool
        nc.vector.tensor_mul(w1[:, 0:3], w1[:, 0:3], bcast(fg[:, 0], 3))
        nc.vector.tensor_mul(w1[:, 3:6], w1[:, 3:6], bcast(fg[:, 0], 3))
        nc.gpsimd.tensor_mul(w1[:, 6:9], w1[:, 6:9], bcast(fg[:, 0], 3))
        # g2 phase: all stencil mults on the DVE (the Pool is busy with the
        # ODD+ product and the w1/w2 folds)
        for j in range(3):
            nc.scalar.activation(out=w2[:, 3 * j:3 * j + 3],
                                 in_=w2[:, 3 * j:3 * j + 3], func=EXP)
        if PREP >= 2:
            norm_chain(1, w2)
        mul_B(0, Rt, xin, 0)
        mm_sum(0, 3, Rt, True, False)
        nc.vector.tensor_mul(p3[:, 8, 0:Rt, :], tap_src(0, 0, Rt, xin),
                             w2[:, 8, 0:Rt, :])  # dil2 center on DVE
        mul_D(0, Rt, xin, 6)
        mul_C2(0, Rt, xin, 3)
        mm_sum(6, 2, Rt, False, False)
        mm_sum(8, 1, Rt, False, False)
        mm_sum(3, 3, Rt, False, True)
        mm_copyout(Rt, lambda r0, rows: p3[:, 10, r0:r0 + rows, :])
        # post-fg1 critical chain: the merged center gates iteration 1's A
        nc.vector.tensor_mul(tmp, w2[:, 8], fg[:, 1])
        nc.vector.tensor_add(w1[:, 1], w1[:, 1], tmp)
        nc.vector.tensor_mul(tm2, tm2, fg[:, 1])
        nc.vector.tensor_add(xout[:, 2:2 + Rt, 2:2 + SW], acc, tm2)
        halo_refresh(xout, Rt)
        # fg1 folds: w2[3:6] next (iteration 1's Pool C group consumes it
        # first), D's planes on the Pool, and w2[0:3] spliced into
        # iteration 1's emission after the A pieces (B consumes it only
        # mid-iteration; emitting it here would stall iteration 1's start)
        nc.vector.tensor_mul(w2[:, 3:6], w2[:, 3:6], bcast(fg[:, 1], 3))
        nc.gpsimd.tensor_mul(w2[:, 6:8], w2[:, 6:8], bcast(fg[:, 1], 2))

    # ---- steady iterations 1..PT-1 on folded planes, row-block pipelined.
    # Per-iteration plane budget: DVE 14 (A incl merged center, ODD-, ODD+,
    # B, D), Pool 3 (C); PE sums all 17 chunk-block-wise.
    for t in range(1, PT):
        Rt = RW - 2 * t
        xin, xout = xb[t % 2], xb[(t + 1) % 2]
        last = (t == PT - 1 and Rt == HH)
        # DVE walks the row pieces of its 5 groups block by block; Pool
        # walks C. Group reach slivers: A/ODD 2, B/D 0, C 3.
        pcs = dict(A=pieces(Rt, 2), O=pieces(Rt, 2), B=pieces(Rt, 0),
                   C=pieces(Rt, 3), D=pieces(Rt, 0))
        # interleave emission row-block-major: all groups' piece i before
        # piece i+1 (per-engine program order then matches data readiness)
        emitters = [
            (pcs["A"], lambda a, b: mul_A(a, b, xin)),
            (pcs["O"], lambda a, b: mul_ODD(a, b, xin, 3, -1)),
            (pcs["O"], lambda a, b: mul_ODD(a, b, xin, 6, +1)),
            (pcs["B"], lambda a, b: mul_B(a, b, xin)),
            (pcs["C"], lambda a, b: mul_C(a, b, xin)),
            (pcs["D"], lambda a, b: mul_D(a, b, xin)),
        ]
        maxp = max(len(p) for p, _ in emitters)
        for i in range(maxp):
            for gi, (plist, fn) in enumerate(emitters):
                if i < len(plist):
                    a, b = plist[i]
                    fn(a, b)
                if t == 1 and i == 0 and gi == 0:
                    # deferred B-plane fold (see iteration 0): after A's
                    # first piece so it cannot stall the iteration start
                    nc.vector.tensor_mul(w2[:, 0:3], w2[:, 0:3],
                                         bcast(fg[:, 1], 3))
        # PE: chunk-major blocks over all 17 planes; copy-out per chunk;
        # halo halves after chunks 1 and 3
        CL = chunks(Rt)
        for ci, (r0, rows) in enumerate(CL):
            subs = [(r0, rows)]
            for sr0, srows in subs:
                first = True
                # Pool's C group last: it is the latest producer per block
                for s, n in ((0, 3), (3, 3), (6, 3), (9, 3), (15, 2), (12, 3)):
                    for j in range(n):
                        nc.tensor.matmul(
                            pacc[ci][:, sr0 - r0:sr0 - r0 + srows], pit,
                            p3[:, s + j, sr0:sr0 + srows, :],
                            start=first, stop=(s + j == 14))
                        first = False
                if last:
                    nc.scalar.copy(out=yc[:, sr0:sr0 + srows, :],
                                   in_=pacc[ci][:, sr0 - r0:sr0 - r0 + srows])
                    nc.sync.dma_start(out=y[:, sr0:sr0 + srows],
                                      in_=yc[:, sr0:sr0 + srows])
                else:
                    nc.scalar.copy(
                        out=xout[:, 2 + sr0:2 + sr0 + srows, 2:2 + SW],
                        in_=pacc[ci][:, 0:srows])
                    if ci == 1:
                        halo_half(xout, 0, min(HB, Rt), 0)
                    elif ci == len(CL) - 1:
                        halo_half(xout, min(HB, Rt), Rt, 2)

    if PT != g["PT"] or PT < 1:
        nc.vector.memset(yc, 0.0)  # PTE diagnostic knob: yc may be unwritten
        nc.sync.dma_start(out=y, in_=yc)
    ctx.close()


# ---------------------------------------------------------------- host side

_FLIPK = np.array([6, 7, 8, 3, 4, 5, 0, 1, 2])
# device plane order (see emit): w1 groups A/ODD-/ODD+, w2 groups B/C/D/ctr
_PERM1 = np.array([1, 4, 7, 0, 3, 6, 2, 5, 8])
_PERM2 = np.array([0, 1, 2, 6, 7, 8, 3, 5, 4])


def _prep_planes(a, half, g, np_dt):
    """a: [K, rows, W] slice -> [K, NS, rows, SW] strip layout (flip if half)."""
    K, rows, W = a.shape
    if half:
        a = a[:, ::-1]
    buf = np.zeros((K, rows, g["Wp"]), dtype=np_dt)
    buf[:, :, :W] = a
    return np.ascontiguousarray(
        buf.reshape(K, rows, g["NS"], g["SW"]).transpose(0, 2, 1, 3))


def host_shard(guided1, guided2, fuse, x, g):
    np_dt = np.dtype(g["dt_name"])
    NS, SW, SWH = g["NS"], g["SW"], g["SWH"]
    RW, RXL, RX, H, W, HH = g["RW"], g["RXL"], g["RX"], g["H"], g["W"], g["HH"]
    pl = np.eye(NS, k=1, dtype=np_dt)
    pr = np.eye(NS, k=-1, dtype=np_dt)
    pi = np.eye(NS, dtype=np_dt)
    cidx = (np.arange(NS) * SW)[:, None] + np.arange(SWH)[None, :]
    in_maps = []
    for c in range(2 * g["B"]):
        b, half = divmod(c, 2)
        wsl = slice(0, RW) if half == 0 else slice(H - RW, H)
        xsl = slice(0, RXL) if half == 0 else slice(H - RXL, H)
        g1p = _prep_planes(guided1[b][:, wsl], half, g, np_dt)
        g2p = _prep_planes(guided2[b][:, wsl], half, g, np_dt)
        if half:
            g1p, g2p = g1p[_FLIPK], g2p[_FLIPK]
        g1p, g2p = g1p[_PERM1], g2p[_PERM2]
        fzp = _prep_planes(fuse[b][:, wsl], half, g, np_dt)
        xa = x[b, 0][xsl]
        if half:
            xa = xa[::-1]
        xp = np.zeros((RX, g["Wp"] + 4), dtype=np_dt)
        xp[2:2 + RXL, 2:2 + W] = xa
        x0 = np.ascontiguousarray(xp[:, cidx].transpose(1, 0, 2))
        in_maps.append(dict(
            g1=np.ascontiguousarray(g1p), g2=np.ascontiguousarray(g2p),
            fz=np.ascontiguousarray(fzp), x0=x0, pl=pl, pr=pr, pi=pi))
    return in_maps


def host_gather(results, g):
    B, H, W, HH, NS, SW = g["B"], g["H"], g["W"], g["HH"], g["NS"], g["SW"]
    out = np.empty((B, 1, H, W), dtype=np.float32)
    for c, res in enumerate(results):
        b, half = divmod(c, 2)
        yimg = res["y"].astype(np.float32).transpose(1, 0, 2).reshape(
            HH, g["Wp"])[:, :W]
        if half:
            out[b, 0, HH:] = yimg[::-1]
        else:
            out[b, 0, :HH] = yimg
    return out


# ---------------------------------------------------------------- build+run

def build(g):
    import concourse.bacc as bacc
    import concourse.mybir as mybir
    import concourse.tile as tile

    DT = getattr(mybir.dt, g["dt_name"])
    NS = g["NS"]
    nc = bacc.Bacc("TRN2", target_bir_lowering=False, debug=False,
                   num_devices=2 * g["B"])
    ins = dict(
        g1=nc.dram_tensor("g1", [9, NS, g["RW"], g["SW"]], DT,
                          kind="ExternalInput").ap(),
        g2=nc.dram_tensor("g2", [9, NS, g["RW"], g["SW"]], DT,
                          kind="ExternalInput").ap(),
        fz=nc.dram_tensor("fz", [2, NS, g["RW"], g["SW"]], DT,
                          kind="ExternalInput").ap(),
        x0=nc.dram_tensor("x0", [NS, g["RX"], g["SWH"]], DT,
                          kind="ExternalInput").ap(),
        pl=nc.dram_tensor("pl", [NS, NS], DT, kind="ExternalInput").ap(),
        pr=nc.dram_tensor("pr", [NS, NS], DT, kind="ExternalInput").ap(),
        pi=nc.dram_tensor("pi", [NS, NS], DT, kind="ExternalInput").ap(),
    )
    outs = dict(
        y=nc.dram_tensor("y", [NS, g["HH"], g["SW"]], DT,
                         kind="ExternalOutput").ap())
    with tile.TileContext(nc) as tc:
        emit(tc, outs, ins, g)
    nc.compile()
    return nc


_CACHE = {}


def _get_nc(g):
    key = tuple(sorted(g.items()))
    if key not in _CACHE:
        _CACHE[key] = build(g)
    return _CACHE[key]


def kernel(guided1, guided2, fuse, x, trace=False):
    from concourse.bass_utils import run_bass_kernel_spmd

    g = make_geom()
    nc = _get_nc(g)
    in_maps = host_shard(
        np.asarray(guided1, dtype=np.float32),
        np.asarray(guided2, dtype=np.float32),
        np.asarray(fuse, dtype=np.float32),
        np.asarray(x, dtype=np.float32), g)
    try:
        res = run_bass_kernel_spmd(nc, in_maps, list(range(2 * g["B"])),
                                   trace=trace)
    except (ImportError, ModuleNotFoundError):
        # NTFF profiling hook unavailable in this container; run untraced
        trace = False
        res = run_bass_kernel_spmd(nc, in_maps, list(range(2 * g["B"])),
                                   trace=False)
    out = host_gather(res.results, g)
    if trace:
        return out, res
    return out


def timeline_estimate_ns():
    """Cost-model (TimelineSim) estimate of per-core device exec time."""
    from concourse.timeline_sim import TimelineSim

    return TimelineSim(_get_nc(make_geom())).simulate()


# revision 60
# speedup vs baseline: 1.0160x; 1.0160x over previous
"""Trainium2 Bass kernel for nn_AffinityPropagate2 (8-iteration dual-dilation
affinity propagation with per-pixel softmax kernels).

Contract: kernel(**inputs) takes FULL numpy inputs
    guided1 [4,9,352,1216] f32, guided2 [4,9,352,1216] f32,
    fuse    [4,2,352,1216] f32, x [4,1,352,1216] f32
and returns the FULL output [4,1,352,1216] f32.

Strategy (8 NeuronCores, SPMD, no cross-core communication):
  - Shard: core c = (batch b = c//2, H-half = c%2). Each core owns 176 output
    rows plus a one-sided ghost zone that shrinks 2 rows per iteration;
    half-1 shards are row-flipped on the host so one SPMD program serves
    all 8 cores.
  - On-chip layout: W padded 1216->1280, 128 column strips of 10
    (partition = strip); H in the free dim. 2 halo columns per side,
    refreshed per iteration via TensorE permutation matmuls.
  - The host pre-permutes the 9 tap planes into tap-GROUP order so each
    fused group is plane-contiguous:
      w1: [A: dh=-1,0,+1 @ dw=0] [ODD-: @ dw=-1] [ODD+: @ dw=+1]
      w2: [B: dh=-2] [C: dh=+2] [D: dh=0,dw=+-2] [center]
  - Softmax folds into 17 per-tap weight planes (dil1+dil2 centers merged
    into w1[1]); zero-padding emulated by zero fuse pad columns.
  - Steady iterations are row-block pipelined: every tap group is split at
    the PSUM-chunk row boundaries (with reach slivers), the DVE walks all
    groups row-block by row-block (14 planes; the Pool/GPSIMD carries the
    3-plane C group), and the PE sums each 48-row chunk of all 17 product
    planes as soon as that row block lands, so the chunk-0 copy-out
    (ScalarE) and the next iteration's first DVE work start mid-iteration
    (the DVE runs gap-free across all 7 steady iterations). Compute dtype
    fp16 (DVE 2x mode).
"""

import numpy as np

# ---------------------------------------------------------------- geometry

def make_geom(B=4, H=352, W=1216, SW=10, NS=128, PT=8, dt_name="float16"):
    HH = H // 2
    g = dict(
        B=B, H=H, W=W, SW=SW, NS=NS, PT=PT, dt_name=dt_name,
        Wp=NS * SW,
        HH=HH,
        RW=HH + 2 * (PT - 1),      # weight rows per shard (incl. ghost)
        RXL=HH + 2 * PT,           # x rows loaded per shard
        SWH=SW + 4,                # strip width incl. 2+2 halo cols
    )
    g["RX"] = g["RXL"] + 4         # x rows incl. 2+2 zero-pad rows
    assert g["Wp"] >= W and NS <= 128
    assert g["RW"] * 2 <= 512, "halo matmul free dim must fit one PSUM bank"
    return g


# ---------------------------------------------------------------- device IR

def emit(tc, outs, ins, g):
    """Emit the SPMD per-core program into TileContext tc.

    ins: dict of DRAM APs: g1 [9,NS,RW,SW], g2 [9,NS,RW,SW] (tap-group
    plane order), fz [2,NS,RW,SW], x0 [NS,RX,SWH], pl/pr/pi [NS,NS]
    outs: y [NS,HH,SW] compute dtype
    """
    import concourse.mybir as mybir
    import concourse.bass as bass_mod

    nc = tc.nc
    NS, SW, SWH, RW, RX, HH, PT = (
        g["NS"], g["SW"], g["SWH"], g["RW"], g["RX"], g["HH"], g["PT"])
    DT = getattr(mybir.dt, g["dt_name"])
    F32 = mybir.dt.float32
    EXP = mybir.ActivationFunctionType.Exp

    g1, g2, fz, x0, pl, pr = (ins[k] for k in ("g1", "g2", "fz", "x0", "pl", "pr"))
    y = outs["y"]

    from contextlib import ExitStack
    ctx = tc.nc._emit_ctx = ExitStack()  # keep pools open until trace ends
    pool = ctx.enter_context(tc.tile_pool(name="main", bufs=1))
    psp = ctx.enter_context(tc.tile_pool(name="ps", bufs=2, space="PSUM"))

    w1 = pool.tile([NS, 9, RW, SW], DT, name="w1", tag="w1")
    w2 = pool.tile([NS, 9, RW, SW], DT, name="w2", tag="w2")
    fg = pool.tile([NS, 2, RW, SW], DT, name="fg", tag="fg")
    xb = [pool.tile([NS, RX, SWH], DT, name=f"xb{i}", tag=f"xb{i}") for i in range(2)]
    # 17 product planes (all live across a pipelined iteration); slots 9:12
    # double as iteration-0's acc/tm2/tmp scratch
    p3 = pool.tile([NS, 17, RW, SW], DT, name="p3", tag="p3")
    r32 = pool.tile([NS, RW, SW], F32, name="r32", tag="r32")
    plt = pool.tile([NS, NS], DT, name="plt", tag="plt")
    prt = pool.tile([NS, NS], DT, name="prt", tag="prt")
    pit = pool.tile([NS, NS], DT, name="pit", tag="pit")
    yc = pool.tile([NS, HH, SW], DT, name="yc", tag="yc")
    acc = p3[:, 9]
    tm2 = p3[:, 10]
    tmp = p3[:, 11]
    CH = 48
    NCH = (RW + CH - 1) // CH
    assert NCH + NCH <= 8 and CH * SW <= 512
    pacc = [psp.tile([NS, CH, SW], F32, name=f"pacc{i}", tag=f"pacc{i}", bufs=1)
            for i in range(NCH)]
    psn = [psp.tile([NS, CH, SW], F32, name=f"psn{i}", tag=f"psn{i}", bufs=1)
           for i in range(NCH)]

    def psn_halo(i, n):
        # [NS, n, 2] f32 view into psn[i]'s bank for the halo matmuls
        base = psn[i][:, 0:1, 0:1]
        return bass_mod.AP(tensor=base.tensor, offset=base.offset,
                           ap=[base.ap[0], [2, n], [1, 2]])

    # ---- loads paced for the ScalarE exp chain (the startup gate): the
    # guided planes stream back-to-back so the exps never starve; perms
    # right before the first psn matmul needs them, x0 before the first
    # stencil mult, fuse before the fg normalizer multiplies
    for k in range(3):
        nc.sync.dma_start(out=w1[:, k], in_=g1[k])
    nc.sync.dma_start(out=plt, in_=pl)
    nc.sync.dma_start(out=prt, in_=pr)
    nc.sync.dma_start(out=pit, in_=ins["pi"])
    nc.sync.dma_start(out=xb[0], in_=x0)
    for k in range(3, 9):
        nc.sync.dma_start(out=w1[:, k], in_=g1[k])
    for k in range(9):
        nc.sync.dma_start(out=w2[:, k], in_=g2[k])
    nc.sync.dma_start(out=fg[:, 0], in_=fz[0])
    nc.sync.dma_start(out=fg[:, 1], in_=fz[1])

    nc.vector.memset(xb[1][:, 0:2, :], 0.0)

    PREP = g.get("PREP_LEVEL", 3)  # perf decomposition only

    def chunks(Rt):
        return [(ci * CH, min(CH, Rt - ci * CH))
                for ci in range((Rt + CH - 1) // CH)]

    # softmax normalizer: fg[s] <- f_s / sum_k exp(g_s[k]). The 9-plane sum
    # rides TensorE identity-matmuls on the psn chunks; the DVE reads the
    # fp32 sums straight out of PSUM for the reciprocal.
    def norm_chain(s, wt):
        for j in range(9):
            for ci, (r0, rows) in enumerate(chunks(RW)):
                nc.tensor.matmul(
                    psn[ci][:, 0:rows], pit, wt[:, j, r0:r0 + rows, :],
                    start=(j == 0), stop=(j == 8))
        # ~51-ULP fp32 reciprocal: far below the fp16 pipeline noise floor
        for ci, (r0, rows) in enumerate(chunks(RW)):
            nc.vector.reciprocal_approx_fast(out=r32[:, r0:r0 + rows, :],
                                             in_=psn[ci][:, 0:rows])
        nc.vector.tensor_mul(fg[:, s], fg[:, s], r32)

    def tap_src(dh, dw, Rt, xin):
        return xin[:, 2 + dh:2 + dh + Rt, 2 + dw:2 + dw + SW]

    def with_dims(base, dims):
        """insert extra leading free dims [step, count] into a sliced AP"""
        return bass_mod.AP(tensor=base.tensor, offset=base.offset,
                           ap=[base.ap[0], *dims, *base.ap[1:]])

    def x_rows(xsrc, row0, col0, dims, a, b):
        base = xsrc[:, row0 + a:row0 + b, col0:col0 + SW]
        return with_dims(base, dims)

    def pieces(Rt, h, first=None):
        """row pieces at the PSUM-chunk boundaries shifted down by the
        group's reach h, so piece c never reads rows beyond what copy-out
        chunks 0..c (and the halo halves) provide; chunk-c sums then wait
        one piece longer, which the gap-free DVE pipeline absorbs"""
        bs = {0, Rt}
        for c in range(1, (Rt + CH - 1) // CH):
            b = c * CH - h
            if 0 < b < Rt:
                bs.add(b)
        if first is not None and 0 < first < Rt:
            bs.add(first)
        bs = sorted(bs)
        return list(zip(bs[:-1], bs[1:]))

    # fused tap-group multiplies — ONE tensor op per 2/3-tap group per row
    # piece. Plane layout is group-contiguous (host pre-permuted):
    #   w1[0:3] A: dil1 dw=0 (w1[1] = merged center)  -> p3[0:3]   DVE
    #   w1[3:6] ODD-: dw=-1 -> p3[3:6]   w1[6:9] ODD+: dw=+1 -> p3[6:9] DVE
    #   w2[0:3] B: dil2 dh=-2 -> p3[9:12]                          DVE
    #   w2[3:6] C: dil2 dh=+2 -> p3[12:15]                         Pool
    #   w2[6:8] D: dil2 dh=0 dw=-+2 -> p3[15:17]                   DVE
    #   w2[8] dil2 center (iteration 0 only; merged into w1[1] after)
    def mul_A(a, b, xin):
        nc.vector.tensor_mul(
            p3[:, 0:3, a:b, :], x_rows(xin, 1, 2, [[SWH, 3]], a, b),
            w1[:, 0:3, a:b, :])

    def mul_ODD(a, b, xin, s, dw):
        nc.vector.tensor_mul(
            p3[:, s:s + 3, a:b, :],
            x_rows(xin, 1, 2 + dw, [[SWH, 3]], a, b),
            w1[:, s:s + 3, a:b, :])

    def mul_B(a, b, xin, s=9):
        nc.vector.tensor_mul(
            p3[:, s:s + 3, a:b, :],
            x_rows(xin, 0, 0, [[2, 3]], a, b), w2[:, 0:3, a:b, :])

    def mul_C(a, b, xin, s=12):  # Pool
        nc.gpsimd.tensor_mul(
            p3[:, s:s + 3, a:b, :],
            x_rows(xin, 4, 0, [[2, 3]], a, b), w2[:, 3:6, a:b, :])

    def mul_C2(a, b, xin, s=12):  # DVE variant (iteration 0)
        nc.vector.tensor_mul(
            p3[:, s:s + 3, a:b, :],
            x_rows(xin, 4, 0, [[2, 3]], a, b), w2[:, 3:6, a:b, :])

    def mul_D(a, b, xin, s=15):
        # first row piece on the Pool: evens DVE (14 planes) vs Pool (3)
        eng = nc.gpsimd if b <= CH else nc.vector
        eng.tensor_mul(
            p3[:, s:s + 2, a:b, :],
            x_rows(xin, 2, 0, [[4, 2]], a, b), w2[:, 6:8, a:b, :])

    def pool_center(Rt, xin, s=8):
        nc.gpsimd.tensor_mul(
            p3[:, s, 0:Rt, :], tap_src(0, 0, Rt, xin), w2[:, 8, 0:Rt, :])

    def mm_sum(s, n, Rt, first, last):
        # plane-major accumulation (iteration 0 only)
        for j in range(n):
            for ci, (r0, rows) in enumerate(chunks(Rt)):
                nc.tensor.matmul(
                    pacc[ci][:, 0:rows], pit,
                    p3[:, s + j, r0:r0 + rows, :],
                    start=(first and j == 0), stop=(last and j == n - 1))

    def mm_copyout_dve(Rt, dst):
        # PSUM f32 -> fp16 SBUF on the DVE (keeps ScalarE free for exps)
        for ci, (r0, rows) in enumerate(chunks(Rt)):
            nc.vector.tensor_copy(out=dst[:, r0:r0 + rows, :],
                                  in_=pacc[ci][:, 0:rows])

    def mm_copyout(Rt, dst_rows_of):
        for ci, (r0, rows) in enumerate(chunks(Rt)):
            nc.scalar.copy(out=dst_rows_of(r0, rows), in_=pacc[ci][:, 0:rows])

    HB = 2 * CH  # halo half boundary

    def halo_half(xout, r0, r1, bi):
        # refresh halo cols for interior rows [r0, r1): left halo cols
        # [0:2) <- neighbor p-1 cols [SW:SW+2) via TensorE permutation
        # matmul (the only cross-partition path in the loop)
        n = r1 - r0
        psl = psn_halo(bi, n)
        nc.tensor.matmul(psl, plt, xout[:, 2 + r0:2 + r1, SW:SW + 2],
                         start=True, stop=True)
        nc.scalar.copy(out=xout[:, 2 + r0:2 + r1, 0:2], in_=psl)
        psr = psn_halo(bi + 1, n)
        nc.tensor.matmul(psr, prt, xout[:, 2 + r0:2 + r1, 2:4],
                         start=True, stop=True)
        nc.scalar.copy(out=xout[:, 2 + r0:2 + r1, SW + 2:SW + 4], in_=psr)

    def halo_refresh(xout, Rt):
        halo_half(xout, 0, min(HB, Rt), 0)
        if Rt > HB:
            halo_half(xout, HB, Rt, 2)

    def bcast(plane_ap, n):
        # broadcast one [NS, rows, SW] plane over n planes via a 0-stride dim
        return bass_mod.AP(tensor=plane_ap.tensor, offset=plane_ap.offset,
                           ap=[plane_ap.ap[0], [0, n], plane_ap.ap[1],
                               plane_ap.ap[2]])

    PT = g.get("PTE", PT)  # emit fewer iterations (perf decomposition only)

    # ---- iteration 0, group-major, interleaved with the load/exp stream.
    # Runs on raw exp planes with per-stencil accumulators:
    #   x1 = acc1*G1 + acc2*G2   (associativity of the softmax fold)
    if PT >= 1:
        Rt = RW
        xin, xout = xb[0], xb[1]
        # g1 phase: DVE takes A + ODD-, Pool takes ODD+ (the Pool is
        # otherwise idle until the w2 exps land)
        for j in range(3):
            nc.scalar.activation(out=w1[:, 3 * j:3 * j + 3],
                                 in_=w1[:, 3 * j:3 * j + 3], func=EXP)
        # norm1 first in the PE stream: its 9-plane psn sum accumulates as
        # the exps land, so only ~3 planes remain after the last exp
        if PREP >= 2:
            norm_chain(0, w1)
        mul_A(0, Rt, xin)
        mm_sum(0, 3, Rt, True, False)
        mul_ODD(0, Rt, xin, 3, -1)
        mm_sum(3, 3, Rt, False, False)
        nc.gpsimd.tensor_mul(
            p3[:, 6:9, 0:Rt, :], x_rows(xin, 1, 3, [[SWH, 3]], 0, Rt),
            w1[:, 6:9, 0:Rt, :])
        mm_sum(6, 3, Rt, False, True)
        # dil1 combine on the DVE's idle window (the ScalarE is mid-exps)
        mm_copyout_dve(Rt, acc)
        nc.vector.tensor_mul(acc, acc, fg[:, 0])
        # w1 folds need only fg0; A/ODD- planes on the DVE (it idles while
        # the w2 exps run), ODD+ planes on the Pool
        nc.vector.tensor_mul(w1[:, 0:3], w1[:, 0:3], bcast(fg[:, 0], 3))
        nc.vector.tensor_mul(w1[:, 3:6], w1[:, 3:6], bcast(fg[:, 0], 3))
        nc.gpsimd.tensor_mul(w1[:, 6:9], w1[:, 6:9], bcast(fg[:, 0], 3))
        # g2 phase: all stencil mults on the DVE (the Pool is busy with the
        # ODD+ product and the w1/w2 folds)
        for j in range(3):
            nc.scalar.activation(out=w2[:, 3 * j:3 * j + 3],
                                 in_=w2[:, 3 * j:3 * j + 3], func=EXP)
        if PREP >= 2:
            norm_chain(1, w2)
        mul_B(0, Rt, xin, 0)
        mm_sum(0, 3, Rt, True, False)
        nc.vector.tensor_mul(p3[:, 8, 0:Rt, :], tap_src(0, 0, Rt, xin),
                             w2[:, 8, 0:Rt, :])  # dil2 center on DVE
        mul_D(0, Rt, xin, 6)
        mul_C2(0, Rt, xin, 3)
        mm_sum(6, 2, Rt, False, False)
        mm_sum(8, 1, Rt, False, False)
        mm_sum(3, 3, Rt, False, True)
        mm_copyout(Rt, lambda r0, rows: p3[:, 10, r0:r0 + rows, :])
        # post-fg1 critical chain: the merged center gates iteration 1's A
        nc.vector.tensor_mul(tmp, w2[:, 8], fg[:, 1])
        nc.vector.tensor_add(w1[:, 1], w1[:, 1], tmp)
        nc.vector.tensor_mul(tm2, tm2, fg[:, 1])
        nc.vector.tensor_add(xout[:, 2:2 + Rt, 2:2 + SW], acc, tm2)
        halo_refresh(xout, Rt)
        # fg1 folds: w2[3:6] next (iteration 1's Pool C group consumes it
        # first), D's planes on the Pool, and w2[0:3] spliced into
        # iteration 1's emission after the A pieces (B consumes it only
        # mid-iteration; emitting it here would stall iteration 1's start)
        nc.vector.tensor_mul(w2[:, 3:6], w2[:, 3:6], bcast(fg[:, 1], 3))
        nc.gpsimd.tensor_mul(w2[:, 6:8], w2[:, 6:8], bcast(fg[:, 1], 2))

    # ---- steady iterations 1..PT-1 on folded planes, row-block pipelined.
    # Per-iteration plane budget: DVE 14 (A incl merged center, ODD-, ODD+,
    # B, D), Pool 3 (C); PE sums all 17 chunk-block-wise.
    for t in range(1, PT):
        Rt = RW - 2 * t
        xin, xout = xb[t % 2], xb[(t + 1) % 2]
        last = (t == PT - 1 and Rt == HH)
        # DVE walks the row pieces of its 5 groups block by block; Pool
        # walks C. Group reach slivers: A/ODD 2, B/D 0, C 3; D keeps a
        # leading CH-row piece for the Pool.
        pcs = dict(A=pieces(Rt, 2), O=pieces(Rt, 2), B=pieces(Rt, 0),
                   C=pieces(Rt, 3), D=pieces(Rt, 0, first=CH))
        # interleave emission row-block-major: all groups' piece i before
        # piece i+1 (per-engine program order then matches data readiness)
        emitters = [
            (pcs["A"], lambda a, b: mul_A(a, b, xin)),
            (pcs["O"], lambda a, b: mul_ODD(a, b, xin, 3, -1)),
            (pcs["O"], lambda a, b: mul_ODD(a, b, xin, 6, +1)),
            (pcs["B"], lambda a, b: mul_B(a, b, xin)),
            (pcs["C"], lambda a, b: mul_C(a, b, xin)),
            (pcs["D"], lambda a, b: mul_D(a, b, xin)),
        ]
        maxp = max(len(p) for p, _ in emitters)
        for i in range(maxp):
            for gi, (plist, fn) in enumerate(emitters):
                if i < len(plist):
                    a, b = plist[i]
                    fn(a, b)
                if t == 1 and i == 0 and gi == 0:
                    # deferred B-plane fold (see iteration 0): after A's
                    # first piece so it cannot stall the iteration start
                    nc.vector.tensor_mul(w2[:, 0:3], w2[:, 0:3],
                                         bcast(fg[:, 1], 3))
        # PE: chunk-major blocks over all 17 planes; copy-out per chunk;
        # halo halves after chunks 1 and 3
        CL = chunks(Rt)
        for ci, (r0, rows) in enumerate(CL):
            subs = [(r0, rows)]
            for sr0, srows in subs:
                first = True
                # Pool's C group last: it is the latest producer per block
                for s, n in ((0, 3), (3, 3), (6, 3), (9, 3), (15, 2), (12, 3)):
                    for j in range(n):
                        nc.tensor.matmul(
                            pacc[ci][:, sr0 - r0:sr0 - r0 + srows], pit,
                            p3[:, s + j, sr0:sr0 + srows, :],
                            start=first, stop=(s + j == 14))
                        first = False
                if last:
                    nc.scalar.copy(out=yc[:, sr0:sr0 + srows, :],
                                   in_=pacc[ci][:, sr0 - r0:sr0 - r0 + srows])
                    nc.sync.dma_start(out=y[:, sr0:sr0 + srows],
                                      in_=yc[:, sr0:sr0 + srows])
                else:
                    nc.scalar.copy(
                        out=xout[:, 2 + sr0:2 + sr0 + srows, 2:2 + SW],
                        in_=pacc[ci][:, 0:srows])
                    if ci == 1:
                        halo_half(xout, 0, min(HB, Rt), 0)
                    elif ci == len(CL) - 1:
                        halo_half(xout, min(HB, Rt), Rt, 2)

    if PT != g["PT"] or PT < 1:
        nc.vector.memset(yc, 0.0)  # PTE diagnostic knob: yc may be unwritten
        nc.sync.dma_start(out=y, in_=yc)
    ctx.close()


# ---------------------------------------------------------------- host side

_FLIPK = np.array([6, 7, 8, 3, 4, 5, 0, 1, 2])
# device plane order (see emit): w1 groups A/ODD-/ODD+, w2 groups B/C/D/ctr
_PERM1 = np.array([1, 4, 7, 0, 3, 6, 2, 5, 8])
_PERM2 = np.array([0, 1, 2, 6, 7, 8, 3, 5, 4])


def _prep_planes(a, half, g, np_dt):
    """a: [K, rows, W] slice -> [K, NS, rows, SW] strip layout (flip if half)."""
    K, rows, W = a.shape
    if half:
        a = a[:, ::-1]
    buf = np.zeros((K, rows, g["Wp"]), dtype=np_dt)
    buf[:, :, :W] = a
    return np.ascontiguousarray(
        buf.reshape(K, rows, g["NS"], g["SW"]).transpose(0, 2, 1, 3))


def host_shard(guided1, guided2, fuse, x, g):
    np_dt = np.dtype(g["dt_name"])
    NS, SW, SWH = g["NS"], g["SW"], g["SWH"]
    RW, RXL, RX, H, W, HH = g["RW"], g["RXL"], g["RX"], g["H"], g["W"], g["HH"]
    pl = np.eye(NS, k=1, dtype=np_dt)
    pr = np.eye(NS, k=-1, dtype=np_dt)
    pi = np.eye(NS, dtype=np_dt)
    cidx = (np.arange(NS) * SW)[:, None] + np.arange(SWH)[None, :]
    in_maps = []
    for c in range(2 * g["B"]):
        b, half = divmod(c, 2)
        wsl = slice(0, RW) if half == 0 else slice(H - RW, H)
        xsl = slice(0, RXL) if half == 0 else slice(H - RXL, H)
        g1p = _prep_planes(guided1[b][:, wsl], half, g, np_dt)
        g2p = _prep_planes(guided2[b][:, wsl], half, g, np_dt)
        if half:
            g1p, g2p = g1p[_FLIPK], g2p[_FLIPK]
        g1p, g2p = g1p[_PERM1], g2p[_PERM2]
        fzp = _prep_planes(fuse[b][:, wsl], half, g, np_dt)
        xa = x[b, 0][xsl]
        if half:
            xa = xa[::-1]
        xp = np.zeros((RX, g["Wp"] + 4), dtype=np_dt)
        xp[2:2 + RXL, 2:2 + W] = xa
        x0 = np.ascontiguousarray(xp[:, cidx].transpose(1, 0, 2))
        in_maps.append(dict(
            g1=np.ascontiguousarray(g1p), g2=np.ascontiguousarray(g2p),
            fz=np.ascontiguousarray(fzp), x0=x0, pl=pl, pr=pr, pi=pi))
    return in_maps


def host_gather(results, g):
    B, H, W, HH, NS, SW = g["B"], g["H"], g["W"], g["HH"], g["NS"], g["SW"]
    out = np.empty((B, 1, H, W), dtype=np.float32)
    for c, res in enumerate(results):
        b, half = divmod(c, 2)
        yimg = res["y"].astype(np.float32).transpose(1, 0, 2).reshape(
            HH, g["Wp"])[:, :W]
        if half:
            out[b, 0, HH:] = yimg[::-1]
        else:
            out[b, 0, :HH] = yimg
    return out


# ---------------------------------------------------------------- build+run

def build(g):
    import concourse.bacc as bacc
    import concourse.mybir as mybir
    import concourse.tile as tile

    DT = getattr(mybir.dt, g["dt_name"])
    NS = g["NS"]
    nc = bacc.Bacc("TRN2", target_bir_lowering=False, debug=False,
                   num_devices=2 * g["B"])
    ins = dict(
        g1=nc.dram_tensor("g1", [9, NS, g["RW"], g["SW"]], DT,
                          kind="ExternalInput").ap(),
        g2=nc.dram_tensor("g2", [9, NS, g["RW"], g["SW"]], DT,
                          kind="ExternalInput").ap(),
        fz=nc.dram_tensor("fz", [2, NS, g["RW"], g["SW"]], DT,
                          kind="ExternalInput").ap(),
        x0=nc.dram_tensor("x0", [NS, g["RX"], g["SWH"]], DT,
                          kind="ExternalInput").ap(),
        pl=nc.dram_tensor("pl", [NS, NS], DT, kind="ExternalInput").ap(),
        pr=nc.dram_tensor("pr", [NS, NS], DT, kind="ExternalInput").ap(),
        pi=nc.dram_tensor("pi", [NS, NS], DT, kind="ExternalInput").ap(),
    )
    outs = dict(
        y=nc.dram_tensor("y", [NS, g["HH"], g["SW"]], DT,
                         kind="ExternalOutput").ap())
    with tile.TileContext(nc) as tc:
        emit(tc, outs, ins, g)
    nc.compile()
    return nc


_CACHE = {}


def _get_nc(g):
    key = tuple(sorted(g.items()))
    if key not in _CACHE:
        _CACHE[key] = build(g)
    return _CACHE[key]


def kernel(guided1, guided2, fuse, x, trace=False):
    from concourse.bass_utils import run_bass_kernel_spmd

    g = make_geom()
    nc = _get_nc(g)
    in_maps = host_shard(
        np.asarray(guided1, dtype=np.float32),
        np.asarray(guided2, dtype=np.float32),
        np.asarray(fuse, dtype=np.float32),
        np.asarray(x, dtype=np.float32), g)
    try:
        res = run_bass_kernel_spmd(nc, in_maps, list(range(2 * g["B"])),
                                   trace=trace)
    except (ImportError, ModuleNotFoundError):
        # NTFF profiling hook unavailable in this container; run untraced
        trace = False
        res = run_bass_kernel_spmd(nc, in_maps, list(range(2 * g["B"])),
                                   trace=False)
    out = host_gather(res.results, g)
    if trace:
        return out, res
    return out


def timeline_estimate_ns():
    """Cost-model (TimelineSim) estimate of per-core device exec time."""
    from concourse.timeline_sim import TimelineSim

    return TimelineSim(_get_nc(make_geom())).simulate()
